# revision 23
# baseline (speedup 1.0000x reference)
"""Max-dilated conv2d kernel for Trainium2 (Bass/Tile), 8-core data parallel.

out[b,oc,oh,ow] = max_{ic,kh,kw} x[b,ic,oh+2*kh, ow+2*kw] * w[oc,ic,kh,kw]

Shapes (hardcoded): x (8,32,68,68) f32, w (32,32,3,3) f32, out (8,32,64,64) f32.
stride=1, dilation=2.

Sharding: batch across the 8 NeuronCores (1 image per core), weights replicated.

mode="tri" (default) — three-engine fp16 pipeline:
  Partition layout p = icq*32 + oc (icq in 0..3, oc in 0..31); the 32 input
  channels form 8 groups of 4 (ic = h*4 + icq).  x is converted to fp16 and
  replicated across the 32 oc partitions ON THE HOST, so the device just
  streams a contiguous [128, 8, 68, 68] fp16 tensor from DRAM (9.2 MB).
  Per (h, kh, kw) plane the work acc = max(acc, x_shifted * w) is split by
  engine at stock-instruction rates:
    - DVE self planes:  tensor_scalar_mul fp16 (4x mode, 0.26 ns/el) into a
      tmp, then tensor_tensor max fp16 (2x mode, 0.52 ns/el) into accD.
    - Act planes: ScalarE computes the product (0.83 ns/el); DVE tensor_max
      folds it into accD.
    - GpSimd planes: ScalarE computes the product; GpSimd tensor_tensor max
      folds it into accG.
  4/2/3 planes per group balance the three engines at ~17 us/group each.
  A cross-partition tree-max (128->64->32, SBUF DMA realign + tensor_max)
  reduces the 4 icq slots; out is written fp16 and cast to fp32 on the host.

mode="fp32"/"mixed" — the previous generation kernel (exact / scalar-offload),
kept for A/B comparison.
"""

import sys

sys.path.insert(0, "/opt/trn_rl_repo")

import numpy as np

import concourse.bacc as bacc
import concourse.tile as tile
from concourse import mybir
from concourse import bass_utils

IC, OC, K = 32, 32, 3
H = W = 68
OH = OW = 64
DH = DW = 2
NCORES = 8
NGROUPS = 8  # ic groups of 4
PLANES = NGROUPS * K * K  # 72
F32 = mybir.dt.float32
F16 = mybir.dt.float16

MODE = "fused"
# mixed mode: how many of the 9 planes per group stay on the exact fp32
# fused-stt path (the rest go ScalarE-fp16-product + VectorE fp16 max)
STT_PER_GROUP = [3, 2, 3, 2, 3, 2, 3, 2]

# tri mode per-group plane routing (k = kh*3+kw in 0..8):
#   D: DVE tensor_scalar_mul + tensor_max   A: ScalarE mul -> DVE max
#   G: ScalarE mul -> GpSimd max
TRI_D = (0, 2, 4, 6)
TRI_A = (7, 8)
TRI_G = (1, 3, 5)

_cache: dict = {}


def _build_tri():
    nc = bacc.Bacc("TRN2", debug=False, num_devices=NCORES)
    xr_d = nc.dram_tensor("xr", [128, NGROUPS, H, W], F16, kind="ExternalInput").ap()
    wv32_d = nc.dram_tensor("wv32", [128, PLANES], F32, kind="ExternalInput").ap()
    out_d = nc.dram_tensor("out", [OC, OH, OW], F16, kind="ExternalOutput").ap()

    # plane routing per group: a planes on the DVE TS path (tensor_scalar_mul
    # 4x fp16 into a tmp), the rest are ScalarE products; every plane is
    # folded into an accumulator on DVE (tensor_max, 2x fp16) via NCH
    # round-robin chains (hides the serial TT write-ack gap).
    A_CNT = [5, 4, 3, 3, 2, 2, 2, 1]  # 22 self planes, 50 ScalarE planes
    # front-loaded: DVE is self-sufficient while ScalarE ramps up

    with tile.TileContext(nc) as tc:
        with (
            tc.tile_pool(name="const", bufs=1) as cpool,
            tc.tile_pool(name="xbuf", bufs=2) as xpool,
            tc.tile_pool(name="pd", bufs=2) as pdpool,
            tc.tile_pool(name="pa", bufs=8) as papool,
            tc.tile_pool(name="work", bufs=1) as wpool,
        ):
            wv32 = cpool.tile([128, PLANES], F32, tag="wv32")
            # weights + group 0 own the DMA subsystem for the first few us;
            # later groups are paced (below) so they don't compete.
            for s4 in range(2):
                p0, p1 = s4 * 64, (s4 + 1) * 64
                nc.scalar.dma_start(wv32[p0:p1, :], wv32_d[p0:p1, :])

            # x tiles rotate through a bufs=4 pool (WAR semaphore also backs
            # off loads if compute falls behind).
            xg: dict = {}

            def load_group(h, eng):
                xg[h] = xpool.tile([128, H, W], F16, tag="xg", name="xg%d" % h)
                if h == 0:
                    # 4 row-chunks so the first planes can start on partial
                    # data (chunk r covers output rows for all kh).
                    for s4 in range(4):
                        r0, r1 = s4 * 17, (s4 + 1) * 17
                        e = [nc.sync, nc.gpsimd, nc.sync, nc.gpsimd][s4]
                        e.dma_start(xg[0][:, r0:r1, :], xr_d[:, 0, r0:r1, :])
                else:
                    eng.dma_start(xg[h][:, :, :], xr_d[:, h])

            # bufs=2 on the x pool paces the stream: group h+2's load DMA
            # carries a WAR wait until group h is fully consumed, so at most
            # ~2 loads are in flight and group 0 + weights land first.
            load_group(0, None)
            load_group(1, nc.sync)

            NCH = 3
            chains = [
                wpool.tile([128, OH, OW], F16, tag="acc%d" % c, name="acc%d" % c)
                for c in range(NCH)
            ]
            chain_live = [False] * NCH
            pending: list = [None] * NCH
            rr = [0]

            def fold(prod_ap):
                c = rr[0] % NCH
                rr[0] += 1
                if not chain_live[c]:
                    if pending[c] is None:
                        pending[c] = prod_ap
                    else:
                        nc.vector.tensor_max(chains[c][:], pending[c], prod_ap)
                        pending[c] = None
                        chain_live[c] = True
                else:
                    nc.vector.tensor_max(chains[c][:], chains[c][:], prod_ap)

            def viewof(h, k, r0=0, r1=OH):
                kh, kw = divmod(k, K)
                return xg[h][
                    :, DH * kh + r0 : DH * kh + r1, DW * kw : DW * kw + OW
                ]

            # group 0's first planes consume the 17-row chunks as they land
            FIRST_SPLITS = [(0, 13), (13, 30), (30, 47), (47, 64)]

            for h in range(NGROUPS):
                if h + 2 < NGROUPS:
                    load_group(h + 2, nc.sync)
                a = A_CNT[h]
                ks = list(range(K * K))
                a_ks, b_ks = ks[:a], ks[a:]
                base = h * (K * K)

                act_prods = []
                for i, k in enumerate(b_ks):
                    j = base + k
                    prod = papool.tile([128, OH, OW], F16, tag="pa")
                    if h == 0 and i < 2:
                        for r0, r1 in FIRST_SPLITS:
                            nc.scalar.mul(
                                prod[:, r0:r1, :],
                                viewof(h, k, r0, r1),
                                wv32[:, j : j + 1],
                            )
                    else:
                        nc.scalar.mul(prod[:], viewof(h, k), wv32[:, j : j + 1])
                    act_prods.append(prod)

                for i, k in enumerate(a_ks):
                    j = base + k
                    c = rr[0] % NCH
                    direct = not chain_live[c] and pending[c] is None
                    dst = chains[c] if direct else pdpool.tile(
                        [128, OH, OW], F16, tag="pd"
                    )
                    if h == 0 and i == 0:
                        for r0, r1 in FIRST_SPLITS:
                            nc.vector.tensor_scalar_mul(
                                dst[:, r0:r1, :],
                                viewof(h, k, r0, r1),
                                wv32[:, j : j + 1],
                            )
                    else:
                        nc.vector.tensor_scalar_mul(
                            dst[:], viewof(h, k), wv32[:, j : j + 1]
                        )
                    if direct:
                        rr[0] += 1
                        chain_live[c] = True
                    else:
                        fold(dst[:])
                for prod in act_prods:
                    fold(prod[:])

            # drain: merge chains into chains[0]
            for c in range(1, NCH):
                assert chain_live[c] and pending[c] is None
                nc.vector.tensor_max(chains[0][:], chains[0][:], chains[c][:])

            # cross-partition tree-max in two pixel halves; reuse chains[1]
            # (dead) as the 64-partition staging and chains[2] as the output
            # staging to save SBUF.
            acc, t64, o32 = chains[0], chains[1], chains[2]
            for hi, (a, b) in enumerate([(0, 32), (32, 64)]):
                for s in range(2):
                    r0 = a + s * 16
                    r1 = r0 + 16
                    eng = nc.gpsimd if s else nc.sync
                    eng.dma_start(t64[0:64, r0:r1, :], acc[64:128, r0:r1, :])
                nc.vector.tensor_max(
                    t64[0:64, a:b, :], t64[0:64, a:b, :], acc[0:64, a:b, :]
                )
                eng = nc.gpsimd if hi else nc.sync
                eng.dma_start(o32[0:32, a:b, :], t64[32:64, a:b, :])
                nc.vector.tensor_max(
                    o32[0:32, a:b, :], o32[0:32, a:b, :], t64[0:32, a:b, :]
                )
                for s in range(2):
                    r0 = a + s * 16
                    r1 = r0 + 16
                    eng = nc.gpsimd if s else nc.sync
                    eng.dma_start(out_d[:, r0:r1, :], o32[0:32, r0:r1, :])

    nc.compile()
    return nc


_MAX_MUL = None


def _register_max_mul():
    """Register a custom DVE op MAX_MUL_ANT: out = max(in0*s0, in1), with a
    hand-authored 2X_1PORT uop program (2 fp16 elems/cycle).  The stock fused
    scalar_tensor_tensor runs at 1 elem/cycle; tensor_tensor at 2 but needs a
    separate product op.  This op does product+max-accumulate in one pass at
    the tensor_tensor rate, making the fold stream self-contained on DVE."""
    global _MAX_MUL
    if _MAX_MUL is not None:
        return _MAX_MUL
    from concourse import dve_ops as dops
    from concourse.dve_spec import Spec, Src0, Src1, C0, maxx, lower
    from concourse.dve_uop import (
        ENABLE,
        AluInp,
        AluOp,
        DelayInp,
        DveOpSpec,
        InpSel,
        OutPath,
        OutSel,
        Trigger,
        UopConfig,
        UopDpConfig,
    )

    spec = Spec(
        body=maxx(Src0 * C0, Src1),
        reference=lambda in0, in1, s0, s1, imm2: np.maximum(
            in0.astype(np.float32) * s0, in1
        ),
    )
    op = dops.DveOp("MAX_MUL_ANT", spec, subdim=False, uops_sha={})
    dops.OPS.append(op)
    dops._SUB_OPCODE_FOR_NAME[op.name] = dops._CUSTOM_DVE_ROW_BASE + len(dops.OPS) - 1
    row = dops._SUB_OPCODE_FOR_NAME[op.name]
    assert row < 0x20

    uops1 = lower(spec, ver="v3")

    # 2X_1PORT program: port reads deliver packed fp16 pairs; lanes
    # SRC_0/SRC_1 carry the low halves, SRC_0_HI/SRC_1_HI the high halves.
    # Slices 0-1 compute the low result, 2-3 the high; low parks in delay
    # lane 1, high rides the ALU chain; write0 packs DELAY_1 | ALU_OUT.
    u = UopConfig()
    u.enable_input(InpSel.SRC_0, 0)      # stage-0 ALU path: x lo
    u.enable_input(InpSel.CONST_0, 1)    # d0: w
    u.enable_input(InpSel.SRC_1, 2)      # d1: acc lo
    u.enable_input(InpSel.SRC_0_HI, 3)   # d2: x hi
    u.enable_input(InpSel.SRC_1_HI, 4)   # d3: acc hi
    dp = u.datapath_config
    dp[0] = (
        UopDpConfig()
        .enable_alu(AluOp.MULTIPLY, AluInp.PREV_ALU_OUT, AluInp.PREV_DELAY_0)
        .pass_through_delay(0, 1, 2, 3)
    )
    dp[1] = (
        UopDpConfig()
        .enable_alu(AluOp.MAX, AluInp.PREV_ALU_OUT, AluInp.PREV_DELAY_1)
        .pass_through_delay(0, 2, 3)
    )
    dp[2] = (
        UopDpConfig()
        .enable_alu(AluOp.MULTIPLY, AluInp.PREV_DELAY_2, AluInp.PREV_DELAY_0)
        .enable_delay_from_src(DelayInp.PREV_ALU_OUT, 1)
        .pass_through_delay(3)
    )
    dp[3] = (
        UopDpConfig()
        .enable_alu(AluOp.MAX, AluInp.PREV_ALU_OUT, AluInp.PREV_DELAY_3)
        .pass_through_delay(1)
    )
    # swap like the stock TT 2x program: LO back onto the ALU chain, HI
    # into delay 0, so the write packer reads ALU_OUT | DELAY_0.
    dp[4] = (
        UopDpConfig()
        .enable_alu(AluOp.BYPASS, AluInp.PREV_DELAY_1, AluInp.PREV_DELAY_1)
        .enable_delay_from_src(DelayInp.PREV_ALU_OUT, 0)
    )
    for k in (5, 6, 7):
        dp[k] = UopDpConfig().pass_through_alu().pass_through_delay(0)
    u.require_inp0 = ENABLE
    u.require_inp1 = ENABLE
    u.trigger = (Trigger.SRC_TENSOR_DONE, Trigger.NONE, Trigger.NONE)
    u.enable_output(OutSel.ALU_OUT, OutPath.WR0_LO)
    u.enable_output(OutSel.DELAY_0, OutPath.WR0_HI)

    spec2 = DveOpSpec(
        name=op.name,
        opcode=row,
        uops=uops1,
        uops_2x=[u],
        perf_max=1,
        rd1_en=True,
    )
    spec2.validate("v3")
    dops._COMPILE_CACHE[(op.name, "v3")] = spec2
    _MAX_MUL = op
    return op


def _build_fused():
    op = _register_max_mul()
    nc = bacc.Bacc("TRN2", debug=False, num_devices=NCORES)
    xr_d = nc.dram_tensor("xr", [128, NGROUPS, H, W], F16, kind="ExternalInput").ap()
    wv32_d = nc.dram_tensor("wv32", [128, PLANES], F32, kind="ExternalInput").ap()
    out_d = nc.dram_tensor("out", [OC, OH, OW], F16, kind="ExternalOutput").ap()

    with tile.TileContext(nc) as tc:
        with (
            tc.tile_pool(name="const", bufs=1) as cpool,
            tc.tile_pool(name="xbuf", bufs=2) as xpool,
            tc.tile_pool(name="work", bufs=1) as wpool,
        ):
            wv32 = cpool.tile([128, PLANES], F32, tag="wv32")
            for s4, e in enumerate([nc.scalar, nc.sync, nc.gpsimd, nc.scalar]):
                p0, p1 = s4 * 32, (s4 + 1) * 32
                e.dma_start(wv32[p0:p1, :], wv32_d[p0:p1, :])

            xg: dict = {}

            def load_group(h, eng):
                xg[h] = xpool.tile([128, H, W], F16, tag="xg", name="xg%d" % h)
                if h == 0:
                    for s4 in range(4):
                        r0, r1 = s4 * 17, (s4 + 1) * 17
                        e = [nc.sync, nc.gpsimd, nc.sync, nc.gpsimd][s4]
                        e.dma_start(xg[0][:, r0:r1, :], xr_d[:, 0, r0:r1, :])
                else:
                    eng.dma_start(xg[h][:, :, :], xr_d[:, h])

            load_group(0, None)
            load_group(1, nc.sync)

            NCH = 3
            chains = [
                wpool.tile([128, OH, OW], F16, tag="acc%d" % c, name="acc%d" % c)
                for c in range(NCH)
            ]
            chain_live = [False] * NCH
            rr = [0]

            def viewof(h, k, r0=0, r1=OH):
                kh, kw = divmod(k, K)
                return xg[h][
                    :, DH * kh + r0 : DH * kh + r1, DW * kw : DW * kw + OW
                ]

            FIRST_SPLITS = [(0, 13), (13, 30), (30, 47), (47, 64)]

            for h in range(NGROUPS):
                if h + 2 < NGROUPS:
                    load_group(h + 2, nc.sync)
                base = h * (K * K)
                for k in range(K * K):
                    j = base + k
                    c = rr[0] % NCH
                    rr[0] += 1
                    wcol = wv32[:, j : j + 1]
                    if not chain_live[c]:
                        # chain init: plain tensor_scalar product (4x mode)
                        if h == 0 and k < 2:
                            for r0, r1 in FIRST_SPLITS:
                                nc.vector.tensor_scalar_mul(
                                    chains[c][:, r0:r1, :],
                                    viewof(h, k, r0, r1),
                                    wcol,
                                )
                        else:
                            nc.vector.tensor_scalar_mul(
                                chains[c][:], viewof(h, k), wcol
                            )
                        chain_live[c] = True
                    else:
                        if h == 0 and k < 4:
                            splits = FIRST_SPLITS
                        elif h == NGROUPS - 1 and k >= K * K - 3:
                            # pixel-split so the tail tree's half A starts
                            # while half B still folds
                            splits = [(0, 32), (32, 64)]
                        else:
                            splits = [(0, OH)]
                        for r0, r1 in splits:
                            nc.vector._custom_dve(
                                op,
                                out=chains[c][:, r0:r1, :],
                                in0=viewof(h, k, r0, r1),
                                in1=chains[c][:, r0:r1, :],
                                s0=wcol,
                            )

            acc, t64, o32 = chains[0], chains[1], chains[2]
            for hi, (a, b) in enumerate([(0, 32), (32, 64)]):
                for c in range(1, NCH):
                    nc.vector.tensor_max(
                        acc[:, a:b, :], acc[:, a:b, :], chains[c][:, a:b, :]
                    )
                for s in range(2):
                    r0 = a + s * 16
                    r1 = r0 + 16
                    eng = nc.gpsimd if s else nc.sync
                    eng.dma_start(t64[0:64, r0:r1, :], acc[64:128, r0:r1, :])
                nc.vector.tensor_max(
                    t64[0:64, a:b, :], t64[0:64, a:b, :], acc[0:64, a:b, :]
                )
                eng = nc.gpsimd if hi else nc.sync
                eng.dma_start(o32[0:32, a:b, :], t64[32:64, a:b, :])
                nc.vector.tensor_max(
                    o32[0:32, a:b, :], o32[0:32, a:b, :], t64[0:32, a:b, :]
                )
                for s in range(2):
                    r0 = a + s * 16
                    r1 = r0 + 16
                    eng = nc.gpsimd if s else nc.sync
                    eng.dma_start(out_d[:, r0:r1, :], o32[0:32, r0:r1, :])

    # enable the 2X_1PORT table slot on every emitted custom-DVE op
    # (byte-36[7:6]; _custom_dve has no perf_max parameter)
    for f in nc.m.functions:
        for blk in f.blocks:
            for i in blk.instructions:
                if hasattr(i, "perf_max") and getattr(i, "op_name", "") == op.name:
                    i.perf_max = 1

    nc.compile()
    return nc


def _build_legacy(mode: str):
    """Previous-generation kernel (fp32 exact / mixed scalar-offload)."""
    nc = bacc.Bacc("TRN2", debug=False, num_devices=NCORES)
    x_d = nc.dram_tensor("x", [IC, H, W], F32, kind="ExternalInput").ap()
    wv_d = nc.dram_tensor("wv", [128, PLANES], F32, kind="ExternalInput").ap()
    out_d = nc.dram_tensor("out", [OC, OH, OW], F32, kind="ExternalOutput").ap()

    mult = mybir.AluOpType.mult
    amax = mybir.AluOpType.max

    with tile.TileContext(nc) as tc:
        with (
            tc.tile_pool(name="const", bufs=1) as cpool,
            tc.tile_pool(name="xrep", bufs=4) as xpool,
            tc.tile_pool(name="work", bufs=1) as wpool,
        ):
            wv_sb = cpool.tile([128, PLANES], F32, tag="wv")
            nc.sync.dma_start(wv_sb[:, :], wv_d[:, :])

            acc_v = wpool.tile([128, OH, OW], F32, tag="acc_v")
            acc_h = (
                wpool.tile([128, OH, OW], F16, tag="acc_h", name="acc_h")
                if mode == "mixed"
                else None
            )

            dma_engines = (
                [nc.sync, nc.scalar, nc.gpsimd]
                if mode == "fp32"
                else [nc.sync, nc.gpsimd]
            )
            first_v = True
            first_h = True
            ei = 0
            for h in range(NGROUPS):
                xr = xpool.tile([128, H, W], F32, tag="xr")
                if h == 0:
                    for s in range(4):
                        r0, r1 = s * 17, (s + 1) * 17
                        for icq in range(4):
                            src = (
                                x_d[h * 4 + icq]
                                .unsqueeze(0)
                                .broadcast_to([32, H, W])
                            )
                            dma_engines[ei % len(dma_engines)].dma_start(
                                xr[icq * 32 : (icq + 1) * 32, r0:r1],
                                src[:, r0:r1],
                            )
                            ei += 1
                else:
                    for icq in range(4):
                        src = (
                            x_d[h * 4 + icq].unsqueeze(0).broadcast_to([32, H, W])
                        )
                        for s in range(2):
                            r0, r1 = s * 34, (s + 1) * 34
                            dma_engines[ei % len(dma_engines)].dma_start(
                                xr[icq * 32 : (icq + 1) * 32, r0:r1],
                                src[:, r0:r1],
                            )
                            ei += 1

                n_stt = K * K if mode == "fp32" else STT_PER_GROUP[h]
                last = h == NGROUPS - 1
                if last:
                    splits = [(0, 32), (32, 64)]
                elif h == 0:
                    splits = None
                else:
                    splits = [(0, 64)]

                for k in range(K * K):
                    kh, kw = divmod(k, K)
                    j = h * (K * K) + k
                    wcol = wv_sb[:, j : j + 1]
                    on_stt = k >= K * K - n_stt
                    if h == 0:
                        if k < 3:
                            ksplits = [(0, 13), (13, 30), (30, 47), (47, 64)]
                        elif k < 6:
                            ksplits = [(0, 30), (30, 64)]
                        else:
                            ksplits = [(0, 64)]
                    else:
                        ksplits = splits
                    for a, b in ksplits:
                        view = xr[
                            :, DH * kh + a : DH * kh + b, DW * kw : DW * kw + OW
                        ]
                        if on_stt:
                            accw = acc_v[:, a:b, :]
                            if first_v:
                                nc.vector.tensor_scalar_mul(accw, view, wcol)
                            else:
                                nc.vector.scalar_tensor_tensor(
                                    accw, view, wcol, accw, mult, amax
                                )
                        else:
                            acch = acc_h[:, a:b, :]
                            if first_h:
                                nc.scalar.mul(acch, view, wcol)
                            else:
                                prod = xpool.tile(
                                    [128, b - a, OW], F16, tag="prod", name="prod", bufs=6
                                )
                                nc.scalar.mul(prod[:], view, wcol)
                                nc.vector.tensor_max(acch, acch, prod[:])
                    if on_stt:
                        first_v = False
                    else:
                        first_h = False

            t64 = wpool.tile([64, OH, OW], F32, tag="t64")
            out_sb = wpool.tile([32, OH, OW], F32, tag="out_sb")
            for hi, (a, b) in enumerate([(0, 32), (32, 64)]):
                if mode == "mixed":
                    nc.vector.tensor_max(
                        acc_v[:, a:b, :], acc_v[:, a:b, :], acc_h[:, a:b, :]
                    )
                for s in range(2):
                    r0 = a + s * 16
                    r1 = r0 + 16
                    dma_engines[(hi + s) % len(dma_engines)].dma_start(
                        t64[:, r0:r1, :], acc_v[64:128, r0:r1, :]
                    )
                nc.vector.tensor_max(
                    t64[:, a:b, :], t64[:, a:b, :], acc_v[0:64, a:b, :]
                )
                dma_engines[hi % len(dma_engines)].dma_start(
                    out_sb[:, a:b, :], t64[32:64, a:b, :]
                )
                nc.vector.tensor_max(
                    out_sb[:, a:b, :], out_sb[:, a:b, :], t64[0:32, a:b, :]
                )
                for s in range(2):
                    r0 = a + s * 16
                    r1 = r0 + 16
                    dma_engines[(hi + s) % len(dma_engines)].dma_start(
                        out_d[:, r0:r1, :], out_sb[:, r0:r1, :]
                    )

    nc.compile()
    return nc


def _build(mode: str = MODE):
    if mode in _cache:
        return _cache[mode]
    if mode == "tri":
        nc = _build_tri()
    elif mode == "fused":
        nc = _build_fused()
    else:
        nc = _build_legacy(mode)
    _cache[mode] = nc
    return nc


def _make_wv(w: np.ndarray) -> np.ndarray:
    """wv[p, h*9+k] = w[p%32, h*4 + p//32, kh, kw] with k = kh*3+kw."""
    wr = w.reshape(OC, NGROUPS, 4, K * K)  # (oc, h, icq, k); ic = h*4+icq
    wv = wr.transpose(2, 0, 1, 3).reshape(4 * OC, PLANES)  # (icq*32+oc, h*9+k)
    return np.ascontiguousarray(wv, dtype=np.float32)


def _make_xrep(x16_b: np.ndarray) -> np.ndarray:
    """[128, 8, 68, 68] fp16 with xrep[icq*32+oc, h] = x16_b[h*4+icq]."""
    xr = x16_b.reshape(NGROUPS, 4, H, W)  # (h, icq, H, W)
    rep = np.broadcast_to(
        xr.transpose(1, 0, 2, 3)[:, None], (4, OC, NGROUPS, H, W)
    ).reshape(128, NGROUPS, H, W)
    return np.ascontiguousarray(rep)


def _ensure_axon_hooks_module():
    """bass_utils imports antenv.axon_hooks when tracing is requested (e.g.
    via BASS_TRACE).  The module is absent on this image; provide a stub so
    the run degrades to untraced instead of crashing."""
    try:
        import antenv.axon_hooks  # noqa: F401
    except Exception:
        import types

        mod = types.ModuleType("antenv.axon_hooks")
        mod._hook = None
        mod.get_axon_ntff_profile_hook = lambda: getattr(mod, "_hook", None)
        mod.set_axon_ntff_profile_hook = lambda h: setattr(mod, "_hook", h)
        sys.modules["antenv.axon_hooks"] = mod
        try:
            import antenv

            antenv.axon_hooks = mod
        except Exception:
            pass


def _make_in_maps(x: np.ndarray, w: np.ndarray, mode: str):
    wv = _make_wv(w)
    if mode in ("tri", "fused"):
        x16 = x.astype(np.float16)
        return [
            {"xr": _make_xrep(x16[b]), "wv32": wv} for b in range(x.shape[0])
        ]
    return [{"x": x[b], "wv": wv} for b in range(x.shape[0])]


def kernel(x, weight, stride_h=1, stride_w=1, dilation_h=2, dilation_w=2):
    _ensure_axon_hooks_module()
    x = np.ascontiguousarray(np.asarray(x, dtype=np.float32))
    w = np.ascontiguousarray(np.asarray(weight, dtype=np.float32))
    assert int(stride_h) == 1 and int(stride_w) == 1
    assert int(dilation_h) == DH and int(dilation_w) == DW
    B = x.shape[0]
    assert x.shape == (B, IC, H, W) and w.shape == (OC, IC, K, K)
    assert B == NCORES

    nc = _build(MODE)
    in_maps = _make_in_maps(x, w, MODE)
    res = bass_utils.run_bass_kernel_spmd(nc, in_maps, core_ids=list(range(B)))
    out = np.stack([r["out"] for r in res.results], axis=0)
    return out.astype(np.float32)


def run_traced(x, weight, mode=MODE, **trace_kwargs):
    """Like kernel() but with hardware profiling; returns (out, BassKernelResults)."""
    x = np.ascontiguousarray(np.asarray(x, dtype=np.float32))
    w = np.ascontiguousarray(np.asarray(weight, dtype=np.float32))
    nc = _build(mode)
    in_maps = _make_in_maps(x, w, mode)
    res = bass_utils.run_bass_kernel_spmd(
        nc, in_maps, core_ids=list(range(x.shape[0])), trace=True, **trace_kwargs
    )
    out = np.stack([r["out"] for r in res.results], axis=0)
    return out.astype(np.float32), res


# revision 24
# speedup vs baseline: 1.0014x; 1.0014x over previous
"""Max-dilated conv2d kernel for Trainium2 (Bass/Tile), 8-core data parallel.

out[b,oc,oh,ow] = max_{ic,kh,kw} x[b,ic,oh+2*kh, ow+2*kw] * w[oc,ic,kh,kw]

Shapes (hardcoded): x (8,32,68,68) f32, w (32,32,3,3) f32, out (8,32,64,64) f32.
stride=1, dilation=2.

Sharding: batch across the 8 NeuronCores (1 image per core), weights replicated.

mode="fused" (default, ~196 us) — custom-DVE single-engine pipeline:
  Partition layout p = icq*32 + oc (icq in 0..3, oc in 0..31); the 32 input
  channels form 8 groups of 4 (ic = h*4 + icq).  x is converted to fp16 and
  replicated across the 32 oc partitions ON THE HOST, so the device streams
  a contiguous [128, 8, 68, 68] fp16 tensor from DRAM (9.2 MB), paced by a
  bufs=2 tile pool so group 0 lands first (one dma_start's descriptors are
  served by one DMA engine; splitting and pacing is what creates overlap).

  The whole reduction runs on the Vector engine via a registered custom DVE
  op MAX_MUL_ANT: acc = max(x*w, acc) in ONE instruction.  The stock fused
  scalar_tensor_tensor runs at 1 fp16 elem/cycle; this op carries a
  hand-authored 2X_1PORT uop-table variant (packed fp16 pairs, slices 0-1
  compute the low element, 2-3 the high, stock tensor_tensor output-packing
  idiom), so it folds at 2 elems/cycle - product and max-accumulate for the
  cost of a plain tensor_max.  72 plane-folds x 4096 px / 2 per cycle at
  0.96 GHz ~= 158 us is the DVE ingest floor; ScalarE/GpSimd cannot help
  (no tensor-max opcode on Pool, DMA CCE has no max) so one engine at the
  floor is optimal.  Three round-robin accumulator chains hide the serial
  write-ack gap; a cross-partition tree-max (128->64->32, SBUF DMA realign +
  tensor_max in two pixel halves) reduces the 4 icq slots; out is written
  fp16 and cast to fp32 on the host.

mode="tri" (~220 us) — stock-op two-engine fallback: DVE self planes
  (tensor_scalar_mul 4x fp16 + tensor_max 2x) and ScalarE product planes
  folded on DVE, 22/50 split.

mode="fp32"/"mixed" (~365/268 us) — the previous generation kernel (exact /
  scalar-offload), kept for A/B comparison.
"""

import sys

sys.path.insert(0, "/opt/trn_rl_repo")

import numpy as np

import concourse.bacc as bacc
import concourse.tile as tile
from concourse import mybir
from concourse import bass_utils

IC, OC, K = 32, 32, 3
H = W = 68
OH = OW = 64
DH = DW = 2
NCORES = 8
NGROUPS = 8  # ic groups of 4
PLANES = NGROUPS * K * K  # 72
F32 = mybir.dt.float32
F16 = mybir.dt.float16

MODE = "fused"
# mixed mode: how many of the 9 planes per group stay on the exact fp32
# fused-stt path (the rest go ScalarE-fp16-product + VectorE fp16 max)
STT_PER_GROUP = [3, 2, 3, 2, 3, 2, 3, 2]

# tri mode per-group plane routing (k = kh*3+kw in 0..8):
#   D: DVE tensor_scalar_mul + tensor_max   A: ScalarE mul -> DVE max
#   G: ScalarE mul -> GpSimd max
TRI_D = (0, 2, 4, 6)
TRI_A = (7, 8)
TRI_G = (1, 3, 5)

_cache: dict = {}


def _build_tri():
    nc = bacc.Bacc("TRN2", debug=False, num_devices=NCORES)
    xr_d = nc.dram_tensor("xr", [128, NGROUPS, H, W], F16, kind="ExternalInput").ap()
    wv32_d = nc.dram_tensor("wv32", [128, PLANES], F32, kind="ExternalInput").ap()
    out_d = nc.dram_tensor("out", [OC, OH, OW], F16, kind="ExternalOutput").ap()

    # plane routing per group: a planes on the DVE TS path (tensor_scalar_mul
    # 4x fp16 into a tmp), the rest are ScalarE products; every plane is
    # folded into an accumulator on DVE (tensor_max, 2x fp16) via NCH
    # round-robin chains (hides the serial TT write-ack gap).
    A_CNT = [5, 4, 3, 3, 2, 2, 2, 1]  # 22 self planes, 50 ScalarE planes
    # front-loaded: DVE is self-sufficient while ScalarE ramps up

    with tile.TileContext(nc) as tc:
        with (
            tc.tile_pool(name="const", bufs=1) as cpool,
            tc.tile_pool(name="xbuf", bufs=2) as xpool,
            tc.tile_pool(name="pd", bufs=2) as pdpool,
            tc.tile_pool(name="pa", bufs=8) as papool,
            tc.tile_pool(name="work", bufs=1) as wpool,
        ):
            wv32 = cpool.tile([128, PLANES], F32, tag="wv32")
            # weights + group 0 own the DMA subsystem for the first few us;
            # later groups are paced (below) so they don't compete.
            for s4 in range(2):
                p0, p1 = s4 * 64, (s4 + 1) * 64
                nc.scalar.dma_start(wv32[p0:p1, :], wv32_d[p0:p1, :])

            # x tiles rotate through a bufs=4 pool (WAR semaphore also backs
            # off loads if compute falls behind).
            xg: dict = {}

            def load_group(h, eng):
                xg[h] = xpool.tile([128, H, W], F16, tag="xg", name="xg%d" % h)
                if h == 0:
                    # 4 row-chunks so the first planes can start on partial
                    # data (chunk r covers output rows for all kh).
                    for s4 in range(4):
                        r0, r1 = s4 * 17, (s4 + 1) * 17
                        e = [nc.sync, nc.gpsimd, nc.sync, nc.gpsimd][s4]
                        e.dma_start(xg[0][:, r0:r1, :], xr_d[:, 0, r0:r1, :])
                else:
                    eng.dma_start(xg[h][:, :, :], xr_d[:, h])

            # bufs=2 on the x pool paces the stream: group h+2's load DMA
            # carries a WAR wait until group h is fully consumed, so at most
            # ~2 loads are in flight and group 0 + weights land first.
            load_group(0, None)
            load_group(1, nc.sync)

            NCH = 3
            chains = [
                wpool.tile([128, OH, OW], F16, tag="acc%d" % c, name="acc%d" % c)
                for c in range(NCH)
            ]
            chain_live = [False] * NCH
            pending: list = [None] * NCH
            rr = [0]

            def fold(prod_ap):
                c = rr[0] % NCH
                rr[0] += 1
                if not chain_live[c]:
                    if pending[c] is None:
                        pending[c] = prod_ap
                    else:
                        nc.vector.tensor_max(chains[c][:], pending[c], prod_ap)
                        pending[c] = None
                        chain_live[c] = True
                else:
                    nc.vector.tensor_max(chains[c][:], chains[c][:], prod_ap)

            def viewof(h, k, r0=0, r1=OH):
                kh, kw = divmod(k, K)
                return xg[h][
                    :, DH * kh + r0 : DH * kh + r1, DW * kw : DW * kw + OW
                ]

            # group 0's first planes consume the 17-row chunks as they land
            FIRST_SPLITS = [(0, 13), (13, 30), (30, 47), (47, 64)]

            for h in range(NGROUPS):
                if h + 2 < NGROUPS:
                    load_group(h + 2, nc.sync)
                a = A_CNT[h]
                ks = list(range(K * K))
                a_ks, b_ks = ks[:a], ks[a:]
                base = h * (K * K)

                act_prods = []
                for i, k in enumerate(b_ks):
                    j = base + k
                    prod = papool.tile([128, OH, OW], F16, tag="pa")
                    if h == 0 and i < 2:
                        for r0, r1 in FIRST_SPLITS:
                            nc.scalar.mul(
                                prod[:, r0:r1, :],
                                viewof(h, k, r0, r1),
                                wv32[:, j : j + 1],
                            )
                    else:
                        nc.scalar.mul(prod[:], viewof(h, k), wv32[:, j : j + 1])
                    act_prods.append(prod)

                for i, k in enumerate(a_ks):
                    j = base + k
                    c = rr[0] % NCH
                    direct = not chain_live[c] and pending[c] is None
                    dst = chains[c] if direct else pdpool.tile(
                        [128, OH, OW], F16, tag="pd"
                    )
                    if h == 0 and i == 0:
                        for r0, r1 in FIRST_SPLITS:
                            nc.vector.tensor_scalar_mul(
                                dst[:, r0:r1, :],
                                viewof(h, k, r0, r1),
                                wv32[:, j : j + 1],
                            )
                    else:
                        nc.vector.tensor_scalar_mul(
                            dst[:], viewof(h, k), wv32[:, j : j + 1]
                        )
                    if direct:
                        rr[0] += 1
                        chain_live[c] = True
                    else:
                        fold(dst[:])
                for prod in act_prods:
                    fold(prod[:])

            # drain: merge chains into chains[0]
            for c in range(1, NCH):
                assert chain_live[c] and pending[c] is None
                nc.vector.tensor_max(chains[0][:], chains[0][:], chains[c][:])

            # cross-partition tree-max in two pixel halves; reuse chains[1]
            # (dead) as the 64-partition staging and chains[2] as the output
            # staging to save SBUF.
            acc, t64, o32 = chains[0], chains[1], chains[2]
            for hi, (a, b) in enumerate([(0, 32), (32, 64)]):
                for s in range(2):
                    r0 = a + s * 16
                    r1 = r0 + 16
                    eng = nc.gpsimd if s else nc.sync
                    eng.dma_start(t64[0:64, r0:r1, :], acc[64:128, r0:r1, :])
                nc.vector.tensor_max(
                    t64[0:64, a:b, :], t64[0:64, a:b, :], acc[0:64, a:b, :]
                )
                eng = nc.gpsimd if hi else nc.sync
                eng.dma_start(o32[0:32, a:b, :], t64[32:64, a:b, :])
                nc.vector.tensor_max(
                    o32[0:32, a:b, :], o32[0:32, a:b, :], t64[0:32, a:b, :]
                )
                for s in range(2):
                    r0 = a + s * 16
                    r1 = r0 + 16
                    eng = nc.gpsimd if s else nc.sync
                    eng.dma_start(out_d[:, r0:r1, :], o32[0:32, r0:r1, :])

    nc.compile()
    return nc


_MAX_MUL = None


def _register_max_mul():
    """Register a custom DVE op MAX_MUL_ANT: out = max(in0*s0, in1), with a
    hand-authored 2X_1PORT uop program (2 fp16 elems/cycle).  The stock fused
    scalar_tensor_tensor runs at 1 elem/cycle; tensor_tensor at 2 but needs a
    separate product op.  This op does product+max-accumulate in one pass at
    the tensor_tensor rate, making the fold stream self-contained on DVE."""
    global _MAX_MUL
    if _MAX_MUL is not None:
        return _MAX_MUL
    from concourse import dve_ops as dops
    from concourse.dve_spec import Spec, Src0, Src1, C0, maxx, lower
    from concourse.dve_uop import (
        ENABLE,
        AluInp,
        AluOp,
        DelayInp,
        DveOpSpec,
        InpSel,
        OutPath,
        OutSel,
        Trigger,
        UopConfig,
        UopDpConfig,
    )

    spec = Spec(
        body=maxx(Src0 * C0, Src1),
        reference=lambda in0, in1, s0, s1, imm2: np.maximum(
            in0.astype(np.float32) * s0, in1
        ),
    )
    op = dops.DveOp("MAX_MUL_ANT", spec, subdim=False, uops_sha={})
    dops.OPS.append(op)
    dops._SUB_OPCODE_FOR_NAME[op.name] = dops._CUSTOM_DVE_ROW_BASE + len(dops.OPS) - 1
    row = dops._SUB_OPCODE_FOR_NAME[op.name]
    assert row < 0x20

    uops1 = lower(spec, ver="v3")

    # 2X_1PORT program: port reads deliver packed fp16 pairs; lanes
    # SRC_0/SRC_1 carry the low halves, SRC_0_HI/SRC_1_HI the high halves.
    # Slices 0-1 compute the low result, 2-3 the high; low parks in delay
    # lane 1, high rides the ALU chain; write0 packs DELAY_1 | ALU_OUT.
    u = UopConfig()
    u.enable_input(InpSel.SRC_0, 0)      # stage-0 ALU path: x lo
    u.enable_input(InpSel.CONST_0, 1)    # d0: w
    u.enable_input(InpSel.SRC_1, 2)      # d1: acc lo
    u.enable_input(InpSel.SRC_0_HI, 3)   # d2: x hi
    u.enable_input(InpSel.SRC_1_HI, 4)   # d3: acc hi
    dp = u.datapath_config
    dp[0] = (
        UopDpConfig()
        .enable_alu(AluOp.MULTIPLY, AluInp.PREV_ALU_OUT, AluInp.PREV_DELAY_0)
        .pass_through_delay(0, 1, 2, 3)
    )
    dp[1] = (
        UopDpConfig()
        .enable_alu(AluOp.MAX, AluInp.PREV_ALU_OUT, AluInp.PREV_DELAY_1)
        .pass_through_delay(0, 2, 3)
    )
    dp[2] = (
        UopDpConfig()
        .enable_alu(AluOp.MULTIPLY, AluInp.PREV_DELAY_2, AluInp.PREV_DELAY_0)
        .enable_delay_from_src(DelayInp.PREV_ALU_OUT, 1)
        .pass_through_delay(3)
    )
    dp[3] = (
        UopDpConfig()
        .enable_alu(AluOp.MAX, AluInp.PREV_ALU_OUT, AluInp.PREV_DELAY_3)
        .pass_through_delay(1)
    )
    # swap like the stock TT 2x program: LO back onto the ALU chain, HI
    # into delay 0, so the write packer reads ALU_OUT | DELAY_0.
    dp[4] = (
        UopDpConfig()
        .enable_alu(AluOp.BYPASS, AluInp.PREV_DELAY_1, AluInp.PREV_DELAY_1)
        .enable_delay_from_src(DelayInp.PREV_ALU_OUT, 0)
    )
    for k in (5, 6, 7):
        dp[k] = UopDpConfig().pass_through_alu().pass_through_delay(0)
    u.require_inp0 = ENABLE
    u.require_inp1 = ENABLE
    u.trigger = (Trigger.SRC_TENSOR_DONE, Trigger.NONE, Trigger.NONE)
    u.enable_output(OutSel.ALU_OUT, OutPath.WR0_LO)
    u.enable_output(OutSel.DELAY_0, OutPath.WR0_HI)

    spec2 = DveOpSpec(
        name=op.name,
        opcode=row,
        uops=uops1,
        uops_2x=[u],
        perf_max=1,
        rd1_en=True,
    )
    spec2.validate("v3")
    dops._COMPILE_CACHE[(op.name, "v3")] = spec2
    _MAX_MUL = op
    return op


def _build_fused():
    op = _register_max_mul()
    nc = bacc.Bacc("TRN2", debug=False, num_devices=NCORES)
    xr_d = nc.dram_tensor("xr", [128, NGROUPS, H, W], F16, kind="ExternalInput").ap()
    wv32_d = nc.dram_tensor("wv32", [128, PLANES], F32, kind="ExternalInput").ap()
    out_d = nc.dram_tensor("out", [OC, OH, OW], F16, kind="ExternalOutput").ap()

    with tile.TileContext(nc) as tc:
        with (
            tc.tile_pool(name="const", bufs=1) as cpool,
            tc.tile_pool(name="xbuf", bufs=2) as xpool,
            tc.tile_pool(name="work", bufs=1) as wpool,
        ):
            wv32 = cpool.tile([128, PLANES], F32, tag="wv32")
            for s4, e in enumerate([nc.scalar, nc.sync, nc.gpsimd, nc.scalar]):
                p0, p1 = s4 * 32, (s4 + 1) * 32
                e.dma_start(wv32[p0:p1, :], wv32_d[p0:p1, :])

            xg: dict = {}

            def load_group(h, eng):
                xg[h] = xpool.tile([128, H, W], F16, tag="xg", name="xg%d" % h)
                if h == 0:
                    for s4 in range(4):
                        r0, r1 = s4 * 17, (s4 + 1) * 17
                        e = [nc.sync, nc.gpsimd, nc.sync, nc.gpsimd][s4]
                        e.dma_start(xg[0][:, r0:r1, :], xr_d[:, 0, r0:r1, :])
                else:
                    eng.dma_start(xg[h][:, :, :], xr_d[:, h])

            load_group(0, None)
            load_group(1, nc.sync)

            NCH = 3
            chains = [
                wpool.tile([128, OH, OW], F16, tag="acc%d" % c, name="acc%d" % c)
                for c in range(NCH)
            ]
            chain_live = [False] * NCH
            rr = [0]

            def viewof(h, k, r0=0, r1=OH):
                kh, kw = divmod(k, K)
                return xg[h][
                    :, DH * kh + r0 : DH * kh + r1, DW * kw : DW * kw + OW
                ]

            FIRST_SPLITS = [(0, 13), (13, 30), (30, 47), (47, 64)]

            for h in range(NGROUPS):
                if h + 2 < NGROUPS:
                    load_group(h + 2, nc.sync)
                base = h * (K * K)
                for k in range(K * K):
                    j = base + k
                    c = rr[0] % NCH
                    rr[0] += 1
                    wcol = wv32[:, j : j + 1]
                    if not chain_live[c]:
                        # chain init: plain tensor_scalar product (4x mode)
                        if h == 0 and k < 2:
                            for r0, r1 in FIRST_SPLITS:
                                nc.vector.tensor_scalar_mul(
                                    chains[c][:, r0:r1, :],
                                    viewof(h, k, r0, r1),
                                    wcol,
                                )
                        else:
                            nc.vector.tensor_scalar_mul(
                                chains[c][:], viewof(h, k), wcol
                            )
                        chain_live[c] = True
                    else:
                        if h == 0 and k < 4:
                            splits = FIRST_SPLITS
                        elif h == NGROUPS - 1 and k >= K * K - 3:
                            # pixel-split so the tail tree's half A starts
                            # while half B still folds
                            splits = [(0, 32), (32, 64)]
                        else:
                            splits = [(0, OH)]
                        for r0, r1 in splits:
                            nc.vector._custom_dve(
                                op,
                                out=chains[c][:, r0:r1, :],
                                in0=viewof(h, k, r0, r1),
                                in1=chains[c][:, r0:r1, :],
                                s0=wcol,
                            )

            acc, t64, o32 = chains[0], chains[1], chains[2]
            for hi, (a, b) in enumerate([(0, 32), (32, 64)]):
                for c in range(1, NCH):
                    nc.vector.tensor_max(
                        acc[:, a:b, :], acc[:, a:b, :], chains[c][:, a:b, :]
                    )
                for s in range(2):
                    r0 = a + s * 16
                    r1 = r0 + 16
                    eng = nc.gpsimd if s else nc.sync
                    eng.dma_start(t64[0:64, r0:r1, :], acc[64:128, r0:r1, :])
                nc.vector.tensor_max(
                    t64[0:64, a:b, :], t64[0:64, a:b, :], acc[0:64, a:b, :]
                )
                eng = nc.gpsimd if hi else nc.sync
                eng.dma_start(o32[0:32, a:b, :], t64[32:64, a:b, :])
                nc.vector.tensor_max(
                    o32[0:32, a:b, :], o32[0:32, a:b, :], t64[0:32, a:b, :]
                )
                for s in range(2):
                    r0 = a + s * 16
                    r1 = r0 + 16
                    eng = nc.gpsimd if s else nc.sync
                    eng.dma_start(out_d[:, r0:r1, :], o32[0:32, r0:r1, :])

    # enable the 2X_1PORT table slot on every emitted custom-DVE op
    # (byte-36[7:6]; _custom_dve has no perf_max parameter)
    for f in nc.m.functions:
        for blk in f.blocks:
            for i in blk.instructions:
                if hasattr(i, "perf_max") and getattr(i, "op_name", "") == op.name:
                    i.perf_max = 1

    nc.compile()
    return nc


def _build_legacy(mode: str):
    """Previous-generation kernel (fp32 exact / mixed scalar-offload)."""
    nc = bacc.Bacc("TRN2", debug=False, num_devices=NCORES)
    x_d = nc.dram_tensor("x", [IC, H, W], F32, kind="ExternalInput").ap()
    wv_d = nc.dram_tensor("wv", [128, PLANES], F32, kind="ExternalInput").ap()
    out_d = nc.dram_tensor("out", [OC, OH, OW], F32, kind="ExternalOutput").ap()

    mult = mybir.AluOpType.mult
    amax = mybir.AluOpType.max

    with tile.TileContext(nc) as tc:
        with (
            tc.tile_pool(name="const", bufs=1) as cpool,
            tc.tile_pool(name="xrep", bufs=4) as xpool,
            tc.tile_pool(name="work", bufs=1) as wpool,
        ):
            wv_sb = cpool.tile([128, PLANES], F32, tag="wv")
            nc.sync.dma_start(wv_sb[:, :], wv_d[:, :])

            acc_v = wpool.tile([128, OH, OW], F32, tag="acc_v")
            acc_h = (
                wpool.tile([128, OH, OW], F16, tag="acc_h", name="acc_h")
                if mode == "mixed"
                else None
            )

            dma_engines = (
                [nc.sync, nc.scalar, nc.gpsimd]
                if mode == "fp32"
                else [nc.sync, nc.gpsimd]
            )
            first_v = True
            first_h = True
            ei = 0
            for h in range(NGROUPS):
                xr = xpool.tile([128, H, W], F32, tag="xr")
                if h == 0:
                    for s in range(4):
                        r0, r1 = s * 17, (s + 1) * 17
                        for icq in range(4):
                            src = (
                                x_d[h * 4 + icq]
                                .unsqueeze(0)
                                .broadcast_to([32, H, W])
                            )
                            dma_engines[ei % len(dma_engines)].dma_start(
                                xr[icq * 32 : (icq + 1) * 32, r0:r1],
                                src[:, r0:r1],
                            )
                            ei += 1
                else:
                    for icq in range(4):
                        src = (
                            x_d[h * 4 + icq].unsqueeze(0).broadcast_to([32, H, W])
                        )
                        for s in range(2):
                            r0, r1 = s * 34, (s + 1) * 34
                            dma_engines[ei % len(dma_engines)].dma_start(
                                xr[icq * 32 : (icq + 1) * 32, r0:r1],
                                src[:, r0:r1],
                            )
                            ei += 1

                n_stt = K * K if mode == "fp32" else STT_PER_GROUP[h]
                last = h == NGROUPS - 1
                if last:
                    splits = [(0, 32), (32, 64)]
                elif h == 0:
                    splits = None
                else:
                    splits = [(0, 64)]

                for k in range(K * K):
                    kh, kw = divmod(k, K)
                    j = h * (K * K) + k
                    wcol = wv_sb[:, j : j + 1]
                    on_stt = k >= K * K - n_stt
                    if h == 0:
                        if k < 3:
                            ksplits = [(0, 13), (13, 30), (30, 47), (47, 64)]
                        elif k < 6:
                            ksplits = [(0, 30), (30, 64)]
                        else:
                            ksplits = [(0, 64)]
                    else:
                        ksplits = splits
                    for a, b in ksplits:
                        view = xr[
                            :, DH * kh + a : DH * kh + b, DW * kw : DW * kw + OW
                        ]
                        if on_stt:
                            accw = acc_v[:, a:b, :]
                            if first_v:
                                nc.vector.tensor_scalar_mul(accw, view, wcol)
                            else:
                                nc.vector.scalar_tensor_tensor(
                                    accw, view, wcol, accw, mult, amax
                                )
                        else:
                            acch = acc_h[:, a:b, :]
                            if first_h:
                                nc.scalar.mul(acch, view, wcol)
                            else:
                                prod = xpool.tile(
                                    [128, b - a, OW], F16, tag="prod", name="prod", bufs=6
                                )
                                nc.scalar.mul(prod[:], view, wcol)
                                nc.vector.tensor_max(acch, acch, prod[:])
                    if on_stt:
                        first_v = False
                    else:
                        first_h = False

            t64 = wpool.tile([64, OH, OW], F32, tag="t64")
            out_sb = wpool.tile([32, OH, OW], F32, tag="out_sb")
            for hi, (a, b) in enumerate([(0, 32), (32, 64)]):
                if mode == "mixed":
                    nc.vector.tensor_max(
                        acc_v[:, a:b, :], acc_v[:, a:b, :], acc_h[:, a:b, :]
                    )
                for s in range(2):
                    r0 = a + s * 16
                    r1 = r0 + 16
                    dma_engines[(hi + s) % len(dma_engines)].dma_start(
                        t64[:, r0:r1, :], acc_v[64:128, r0:r1, :]
                    )
                nc.vector.tensor_max(
                    t64[:, a:b, :], t64[:, a:b, :], acc_v[0:64, a:b, :]
                )
                dma_engines[hi % len(dma_engines)].dma_start(
                    out_sb[:, a:b, :], t64[32:64, a:b, :]
                )
                nc.vector.tensor_max(
                    out_sb[:, a:b, :], out_sb[:, a:b, :], t64[0:32, a:b, :]
                )
                for s in range(2):
                    r0 = a + s * 16
                    r1 = r0 + 16
                    dma_engines[(hi + s) % len(dma_engines)].dma_start(
                        out_d[:, r0:r1, :], out_sb[:, r0:r1, :]
                    )

    nc.compile()
    return nc


def _build(mode: str = MODE):
    if mode in _cache:
        return _cache[mode]
    if mode == "tri":
        nc = _build_tri()
    elif mode == "fused":
        nc = _build_fused()
    else:
        nc = _build_legacy(mode)
    _cache[mode] = nc
    return nc


def _make_wv(w: np.ndarray) -> np.ndarray:
    """wv[p, h*9+k] = w[p%32, h*4 + p//32, kh, kw] with k = kh*3+kw."""
    wr = w.reshape(OC, NGROUPS, 4, K * K)  # (oc, h, icq, k); ic = h*4+icq
    wv = wr.transpose(2, 0, 1, 3).reshape(4 * OC, PLANES)  # (icq*32+oc, h*9+k)
    return np.ascontiguousarray(wv, dtype=np.float32)


def _make_xrep(x16_b: np.ndarray) -> np.ndarray:
    """[128, 8, 68, 68] fp16 with xrep[icq*32+oc, h] = x16_b[h*4+icq]."""
    xr = x16_b.reshape(NGROUPS, 4, H, W)  # (h, icq, H, W)
    rep = np.broadcast_to(
        xr.transpose(1, 0, 2, 3)[:, None], (4, OC, NGROUPS, H, W)
    ).reshape(128, NGROUPS, H, W)
    return np.ascontiguousarray(rep)


def _ensure_axon_hooks_module():
    """bass_utils imports antenv.axon_hooks when tracing is requested (e.g.
    via BASS_TRACE).  The module is absent on this image; provide a stub so
    the run degrades to untraced instead of crashing."""
    try:
        import antenv.axon_hooks  # noqa: F401
    except Exception:
        import types

        mod = types.ModuleType("antenv.axon_hooks")
        mod._hook = None
        mod.get_axon_ntff_profile_hook = lambda: getattr(mod, "_hook", None)
        mod.set_axon_ntff_profile_hook = lambda h: setattr(mod, "_hook", h)
        sys.modules["antenv.axon_hooks"] = mod
        try:
            import antenv

            antenv.axon_hooks = mod
        except Exception:
            pass


def _make_in_maps(x: np.ndarray, w: np.ndarray, mode: str):
    wv = _make_wv(w)
    if mode in ("tri", "fused"):
        x16 = x.astype(np.float16)
        return [
            {"xr": _make_xrep(x16[b]), "wv32": wv} for b in range(x.shape[0])
        ]
    return [{"x": x[b], "wv": wv} for b in range(x.shape[0])]


def kernel(x, weight, stride_h=1, stride_w=1, dilation_h=2, dilation_w=2):
    _ensure_axon_hooks_module()
    x = np.ascontiguousarray(np.asarray(x, dtype=np.float32))
    w = np.ascontiguousarray(np.asarray(weight, dtype=np.float32))
    assert int(stride_h) == 1 and int(stride_w) == 1
    assert int(dilation_h) == DH and int(dilation_w) == DW
    B = x.shape[0]
    assert x.shape == (B, IC, H, W) and w.shape == (OC, IC, K, K)
    assert B == NCORES

    nc = _build(MODE)
    in_maps = _make_in_maps(x, w, MODE)
    res = bass_utils.run_bass_kernel_spmd(nc, in_maps, core_ids=list(range(B)))
    out = np.stack([r["out"] for r in res.results], axis=0)
    return out.astype(np.float32)


def run_traced(x, weight, mode=MODE, **trace_kwargs):
    """Like kernel() but with hardware profiling; returns (out, BassKernelResults)."""
    x = np.ascontiguousarray(np.asarray(x, dtype=np.float32))
    w = np.ascontiguousarray(np.asarray(weight, dtype=np.float32))
    nc = _build(mode)
    in_maps = _make_in_maps(x, w, mode)
    res = bass_utils.run_bass_kernel_spmd(
        nc, in_maps, core_ids=list(range(x.shape[0])), trace=True, **trace_kwargs
    )
    out = np.stack([r["out"] for r in res.results], axis=0)
    return out.astype(np.float32), res
